# revision 10
# baseline (speedup 1.0000x reference)
"""Trainium2 Bass kernel for nn_EnsembleE2EModule (moe_routing).

Reference computation (B=4096, D=784, C=10, E=1024, K=8):
  cos  = l2norm(x) @ keys.T                    [B, E]
  sims, idx = top_k(cos, 8)  (descending sims)
  gidx = sort(idx)           (ascending expert ids)
  expert_out = tanh((x @ Wm[gidx].T + bm[gidx]) / 10) * 10   [B, K, C]
  ensemble = sum_k sims_k * expert_out_k / sum_k sims_k      [B, C]
  tanh_out = tanh((x @ Wt.T + bt) / 10) * 10                 [B, C]
  vanilla  = log_softmax(x @ Wv.T + bv)                      [B, C]

Sharding: data-parallel over B across 8 NeuronCores (512 rows each);
keys / expert stack / classifier weights replicated on every core.

Since sims appear in both numerator and denominator of the ensemble,
the 1/||x|| row scaling cancels — top-k is computed on raw dot
products (same order as cosine) and the raw dot values are used as
weights directly; no normalization pass is needed.
"""

import numpy as np

import concourse.bass as bass
import concourse.bacc as bacc
import concourse.tile as tile
import concourse.mybir as mybir
from concourse.masks import make_identity

f32 = mybir.dt.float32
u32 = mybir.dt.uint32
AF = mybir.ActivationFunctionType
ALU = mybir.AluOpType
AX = mybir.AxisListType

B, D, C, E, K = 4096, 784, 10, 1024, 8
N_CORES = 8
B_SH = B // N_CORES          # 512 rows per core
P = 128                      # SBUF partitions
N_TILES = B_SH // P          # 4 sample tiles per core
DC = 112                     # contraction chunk (784 = 7 * 112)
N_CH = D // DC               # 7 chunks
ROW = C * D                  # 7840 floats of gathered weights per expert
ROWB = ROW + C               # + C bias floats appended per expert


def build_kernel(nc: bass.Bass, reps: int = 1):
    """Emit the per-core Tile program. Core-agnostic: each core gets its own
    x shard via in_maps; weights are replicated. reps>1 repeats the whole
    body (timing only: t(reps=2)-t(reps=1) cancels dispatch overhead)."""
    x_d = nc.dram_tensor("x_sh", [B_SH, D], f32, kind="ExternalInput")
    keys_d = nc.dram_tensor("keys", [E, D], f32, kind="ExternalInput")
    wcat_d = nc.dram_tensor("wcat", [E, ROWB], f32, kind="ExternalInput")
    wv_d = nc.dram_tensor("wv", [C, D], f32, kind="ExternalInput")
    bv_d = nc.dram_tensor("bv", [1, C], f32, kind="ExternalInput")
    wt_d = nc.dram_tensor("wt", [C, D], f32, kind="ExternalInput")
    bt_d = nc.dram_tensor("bt", [1, C], f32, kind="ExternalInput")

    ens_d = nc.dram_tensor("ens", [B_SH, C], f32, kind="ExternalOutput")
    tnh_d = nc.dram_tensor("tnh", [B_SH, C], f32, kind="ExternalOutput")
    van_d = nc.dram_tensor("van", [B_SH, C], f32, kind="ExternalOutput")

    with tile.TileContext(nc) as tc:
        with (
            tc.tile_pool(name="const", bufs=1) as cpool,
            tc.tile_pool(name="ktr", bufs=2) as kpool,
            tc.tile_pool(name="xio", bufs=2) as xpool,
            tc.tile_pool(name="route", bufs=2) as rpool,
            tc.tile_pool(name="gath", bufs=3) as gpool,
            tc.tile_pool(name="small", bufs=2) as spool,
            tc.tile_pool(name="ps_t", bufs=2, space="PSUM") as ps_t,
            tc.tile_pool(name="ps_cos", bufs=2, space="PSUM") as ps_cos,
            tc.tile_pool(name="ps_cls", bufs=2, space="PSUM") as ps_cls,
        ):
          for _rep in range(reps):
            ident = cpool.tile([P, P], f32, tag="ident")
            make_identity(nc, ident[:])
            ones_row = cpool.tile([1, P], f32)
            nc.vector.memset(ones_row[:], 1.0)
            bv_t = cpool.tile([1, C], f32)
            nc.sync.dma_start(bv_t[:], bv_d[:])
            bt_t = cpool.tile([1, C], f32)
            nc.sync.dma_start(bt_t[:], bt_d[:])

            # ---- transpose keys -> keysT chunks [DC, E] (7 tiles) ----
            keysT = [
                cpool.tile([DC, E], f32, name=f"keysT{c}", tag=f"keysT{c}")
                for c in range(N_CH)
            ]
            for b in range(E // P):
                kblk = kpool.tile([P, D], f32, tag="kblk")
                nc.sync.dma_start(kblk[:], keys_d[b * P:(b + 1) * P, :])
                for c in range(N_CH):
                    pt = ps_t.tile([DC, P], f32, tag="ptr")
                    nc.tensor.transpose(pt[:], kblk[:, c * DC:(c + 1) * DC], ident[:])
                    nc.scalar.copy(keysT[c][:, b * P:(b + 1) * P], pt[:])

            # ---- transpose classifier weights -> [DC, C] chunks ----
            wvT = cpool.tile([DC, N_CH * C], f32)
            wtT = cpool.tile([DC, N_CH * C], f32)
            for (w_d, wT) in ((wv_d, wvT), (wt_d, wtT)):
                wblk = kpool.tile([C, D], f32, tag="wblk")
                nc.sync.dma_start(wblk[:], w_d[:])
                for c in range(N_CH):
                    pt = ps_t.tile([DC, C], f32, tag="ptrc")
                    nc.tensor.transpose(
                        pt[:], wblk[:, c * DC:(c + 1) * DC], ident[:C, :C]
                    )
                    nc.scalar.copy(wT[:, c * C:(c + 1) * C], pt[:])

            # ---- per sample-tile pipeline ----
            for t in range(N_TILES):
                x_t = xpool.tile([P, D], f32, tag="x")
                nc.sync.dma_start(x_t[:], x_d[t * P:(t + 1) * P, :])

                # transpose x tile -> xT chunks [DC, P]
                xT = xpool.tile([DC, N_CH * P], f32, tag="xT")
                for c in range(N_CH):
                    pt = ps_t.tile([DC, P], f32, tag="ptr")
                    nc.tensor.transpose(pt[:], x_t[:, c * DC:(c + 1) * DC], ident[:])
                    nc.scalar.copy(xT[:, c * P:(c + 1) * P], pt[:])

                # cos = x @ keys.T  (raw dots; row scaling cancels)
                cos_t = rpool.tile([P, E], f32, tag="cos")
                for h in range(2):
                    pc = ps_cos.tile([P, E // 2], f32, tag="pcos")
                    for c in range(N_CH):
                        nc.tensor.matmul(
                            pc[:],
                            lhsT=xT[:, c * P:(c + 1) * P],
                            rhs=keysT[c][:, h * (E // 2):(h + 1) * (E // 2)],
                            start=(c == 0),
                            stop=(c == N_CH - 1),
                        )
                    nc.vector.tensor_copy(cos_t[:, h * (E // 2):(h + 1) * (E // 2)], pc[:])

                # top-8 (descending) + indices
                w_t = rpool.tile([P, K], f32, tag="w")
                idx_t = rpool.tile([P, K], u32, tag="idx")
                nc.vector.max(out=w_t[:], in_=cos_t[:])
                nc.vector.max_index(out=idx_t[:], in_max=w_t[:], in_values=cos_t[:])

                # ascending expert ids: u32 -> f32, max8 (desc), reverse, -> u32
                idxf = rpool.tile([P, K], f32, tag="idxf")
                nc.vector.tensor_copy(idxf[:], idx_t[:])
                dsc = rpool.tile([P, K], f32, tag="dsc")
                nc.vector.max(out=dsc[:], in_=idxf[:])
                asc = rpool.tile([P, K], u32, tag="asc")
                nc.vector.tensor_copy(asc[:], dsc[:, ::-1])

                # gather expert weight rows and run the per-pair GEMVs:
                # prod = x (bcast over classes) * Wg on DVE, then 10 ACT
                # copy-accumulates reduce each class row into raw[:, c*K+k].
                # (tensor_tensor_reduce is avoided: broken on this runtime.)
                raw = spool.tile([P, C * K], f32, tag="raw")  # [P, c-major, k-minor]
                for k in range(K):
                    wg = gpool.tile([P, ROWB], f32, tag="wg", bufs=2)
                    nc.gpsimd.indirect_dma_start(
                        out=wg[:],
                        out_offset=None,
                        in_=wcat_d[:],
                        in_offset=bass.IndirectOffsetOnAxis(ap=asc[:, k:k + 1], axis=0),
                    )
                    prod = gpool.tile([P, ROW], f32, tag="prod", bufs=2)
                    nc.vector.tensor_tensor(
                        out=prod[:].rearrange("p (c d) -> p c d", c=C),
                        in0=x_t[:].unsqueeze(1).to_broadcast([P, C, D]),
                        in1=wg[:, :ROW].rearrange("p (c d) -> p c d", c=C),
                        op=ALU.mult,
                    )
                    for c in range(C):
                        nc.scalar.activation(
                            out=prod[:, c * D:(c + 1) * D],
                            in_=prod[:, c * D:(c + 1) * D],
                            func=AF.Copy,
                            accum_out=raw[:, c * K + k:c * K + k + 1],
                        )
                    # add gathered per-expert bias to this k's column slice
                    nc.vector.tensor_add(
                        raw[:, k::K], raw[:, k::K], wg[:, ROW:ROW + C]
                    )

                # expert_out = tanh(raw/10); ensemble = 10*sum_k w*eo / sum_k w
                eo = spool.tile([P, C * K], f32, tag="eo")
                nc.scalar.activation(eo[:], raw[:], AF.Tanh, scale=0.1)
                prod = spool.tile([P, C * K], f32, tag="prod")
                nc.vector.tensor_tensor(
                    out=prod[:].rearrange("p (c k) -> p c k", c=C),
                    in0=w_t[:].unsqueeze(1).to_broadcast([P, C, K]),
                    in1=eo[:].rearrange("p (c k) -> p c k", c=C),
                    op=ALU.mult,
                )
                ens_num = spool.tile([P, C], f32, tag="ensn")
                nc.vector.tensor_reduce(
                    out=ens_num[:],
                    in_=prod[:].rearrange("p (c k) -> p c k", c=C),
                    axis=AX.X,
                    op=ALU.add,
                )
                wsum = spool.tile([P, 1], f32, tag="wsum")
                nc.vector.tensor_reduce(out=wsum[:], in_=w_t[:], axis=AX.X, op=ALU.add)
                nc.vector.tensor_scalar_mul(wsum[:], wsum[:], 0.1)
                winv = spool.tile([P, 1], f32, tag="winv")
                nc.vector.reciprocal(winv[:], wsum[:])
                ens_t = spool.tile([P, C], f32, tag="ens")
                nc.vector.tensor_scalar_mul(ens_t[:], ens_num[:], winv[:, 0:1])
                nc.sync.dma_start(ens_d[t * P:(t + 1) * P, :], ens_t[:])

                # ---- classifiers ----
                for which, (wT, b_t, out_d) in enumerate(
                    ((wvT, bv_t, van_d), (wtT, bt_t, tnh_d))
                ):
                    pl = ps_cls.tile([P, C], f32, tag="pcls")
                    for c in range(N_CH):
                        nc.tensor.matmul(
                            pl[:],
                            lhsT=xT[:, c * P:(c + 1) * P],
                            rhs=wT[:, c * C:(c + 1) * C],
                            start=(c == 0),
                            stop=False,
                        )
                    nc.tensor.matmul(
                        pl[:], lhsT=ones_row[:], rhs=b_t[:], start=False, stop=True
                    )
                    logits = spool.tile([P, C], f32, tag=f"log{which}")
                    nc.vector.tensor_copy(logits[:], pl[:])
                    if which == 1:
                        # tanh_out = tanh(logits/10)*10
                        th = spool.tile([P, C], f32, tag="th")
                        nc.scalar.activation(th[:], logits[:], AF.Tanh, scale=0.1)
                        out_t = spool.tile([P, C], f32, tag="tout")
                        nc.vector.tensor_scalar_mul(out_t[:], th[:], 10.0)
                    else:
                        # vanilla = log_softmax(logits)
                        mx = spool.tile([P, 1], f32, tag="mx")
                        nc.vector.tensor_reduce(
                            out=mx[:], in_=logits[:], axis=AX.X, op=ALU.max
                        )
                        sh = spool.tile([P, C], f32, tag="sh")
                        nc.vector.tensor_scalar(
                            out=sh[:], in0=logits[:], scalar1=mx[:, 0:1],
                            scalar2=None, op0=ALU.subtract,
                        )
                        ex = spool.tile([P, C], f32, tag="ex")
                        se = spool.tile([P, 1], f32, tag="se")
                        nc.scalar.activation(ex[:], sh[:], AF.Exp, accum_out=se[:])
                        lse = spool.tile([P, 1], f32, tag="lse")
                        nc.scalar.activation(lse[:], se[:], AF.Ln)
                        out_t = spool.tile([P, C], f32, tag="vout")
                        nc.vector.tensor_scalar(
                            out=out_t[:], in0=sh[:], scalar1=lse[:, 0:1],
                            scalar2=None, op0=ALU.subtract,
                        )
                    nc.sync.dma_start(out_d[t * P:(t + 1) * P, :], out_t[:])

    nc.finalize()
    return nc


def make_in_maps(x, keys, Wm, bm, Wv, bv, Wt, bt):
    """Host-side marshalling only: shard x over cores, replicate weights,
    concatenate Wm|bm into the per-expert gather rows (pure layout)."""
    wcat = np.concatenate(
        [np.ascontiguousarray(Wm, np.float32).reshape(E, ROW),
         np.ascontiguousarray(bm, np.float32)], axis=1)
    common = dict(
        keys=np.ascontiguousarray(keys, np.float32),
        wcat=wcat,
        wv=np.ascontiguousarray(Wv, np.float32),
        bv=np.ascontiguousarray(bv, np.float32).reshape(1, C),
        wt=np.ascontiguousarray(Wt, np.float32),
        bt=np.ascontiguousarray(bt, np.float32).reshape(1, C),
    )
    x = np.ascontiguousarray(x, np.float32)
    return [
        dict(x_sh=x[c * B_SH:(c + 1) * B_SH], **common) for c in range(N_CORES)
    ]


_CACHED = {}


def _get_nc(reps: int = 1):
    key = f"nc{reps}"
    if key not in _CACHED:
        nc = bacc.Bacc(debug=False)
        build_kernel(nc, reps=reps)
        _CACHED[key] = nc
    return _CACHED[key]


def kernel(x, keys, Wm, bm, Wv, bv, Wt, bt):
    from concourse.bass_utils import run_bass_kernel_spmd

    nc = _get_nc()
    in_maps = make_in_maps(x, keys, Wm, bm, Wv, bv, Wt, bt)
    res = run_bass_kernel_spmd(nc, in_maps, core_ids=list(range(N_CORES))).results
    ensemble = np.concatenate([res[c]["ens"] for c in range(N_CORES)], axis=0)
    tanh_out = np.concatenate([res[c]["tnh"] for c in range(N_CORES)], axis=0)
    vanilla = np.concatenate([res[c]["van"] for c in range(N_CORES)], axis=0)
    return ensemble, tanh_out, vanilla
